# revision 13
# baseline (speedup 1.0000x reference)
"""Trainium2 Bass kernel for nn_CAGKE_1 (Gaussian-kernel embedding).

Math: reference computes, for mask m_i = 1[X_i > 0.5],
    out[j] = sum_e softmax(w)_e * sum_i m_i * (c/sigma_e) exp(-(j-i-1)^2/(2 sigma_e^2)) + noise_j
Both sums are linear, so the E=128 channels collapse into one combined
kernel ghat(d) = sum_e softmax(w)_e * (c/sigma_e) exp(-d^2/(2 sigma_e^2))
BEFORE the convolution. With sigma <= 5, taps |d| > 16 carry < 2e-4
relative mass, so a 32-tap kernel (d in [-16, 15]) is far inside the
2e-2 gate (measured ~1.3e-3 Frobenius with bf16 matmuls).

Per core (1024 outputs, no collectives, no DRAM round-trips, no
transposes):
  - The conv is ONE bf16 matmul: stationary gblk [128, 4] holds the
    unnormalized ghat four times block-diagonally (rows 32q..32q+31 ->
    col q); moving operand is the binarized mask window [128, 256]
    whose partition k = 32q+t' holds X[c*1024 + 256q + t' : +256]
    (host-layout overlapping windows = mask-side Toeplitz, 4 stacked
    blocks) -> out_ps[q, j'] ~ out[256q + j'].
  - ghat column: expt[e, k] = c*exp(-d(k)^2/(2 sigma_e^2)) (ACT,
    per-partition scale (1/sigma)^2, ln(c) bias, tap grid tiled 4x
    along the free axis, d(k) = 15 - (k mod 32)) matmul'd against
    a = exp(w)/sigma, computed directly as a column (w arrives as a
    column next to sigma - no PE transpose, no row softmax).
  - softmax normalization is deferred to the end: Z replicated to
    PSUM [4,1] by a tiny ones[128,4]^T @ exp(w) matmul, one DVE
    reciprocal, then the single PSUM evacuation op computes
    out = out_ps * (1/Z) + noise.
DMA plan: mask windows alone on Sync HWDGE (land first), sigma|w
column then noise on Scalar HWDGE; one [4, 256] store at the end.
"""

import sys

import numpy as np

if "/opt/trn_rl_repo" not in sys.path:
    sys.path.insert(0, "/opt/trn_rl_repo")

T = 8192
E = 128
N_CORES = 8
TJ = T // N_CORES          # 1024 outputs per core
Q = 4                      # output blocks per core (psum rows)
JW = TJ // Q               # 256 outputs per block
KT = 128 // Q              # 32 taps per block
DMAX = 15                  # d = DMAX - t', t' in [0, KT) -> d in [-16, 15]
INV_SQRT_2PI = 0.39894228
LNC = float(np.log(np.float32(INV_SQRT_2PI)))

_compiled = None


def _build():
    import concourse.bacc as bacc
    import concourse.mybir as mybir
    import concourse.tile as tile

    f32 = mybir.dt.float32
    bf16 = mybir.dt.bfloat16
    nc = bacc.Bacc(num_devices=N_CORES, debug=False)

    mw_d = nc.dram_tensor("mw", [128, JW], f32, kind="ExternalInput")
    sgw_d = nc.dram_tensor("sgw", [128, 2], f32, kind="ExternalInput")
    wn_d = nc.dram_tensor("wn", [Q, JW], f32, kind="ExternalInput")
    out_d = nc.dram_tensor("out", [Q, JW], f32, kind="ExternalOutput")

    with tile.TileContext(nc) as tc:
        if True:
            # ---- latency-critical input loads ----
            mw = nc.alloc_sbuf_tensor("mw_s", [128, JW], f32)
            nc.sync.dma_start(mw[:], mw_d[:])
            sgw = nc.alloc_sbuf_tensor("sgw_s", [128, 2], f32)
            nc.scalar.dma_start(sgw[:], sgw_d[:])
            wn = nc.alloc_sbuf_tensor("wn_s", [Q, JW], f32)
            nc.scalar.dma_start(wn[:], wn_d[:])
            sg_col = sgw[:, 0:1]
            w_col = sgw[:, 1:2]

            # ---- input-independent prep (off critical path) ----
            dum = nc.alloc_sbuf_tensor("dum", [1, 1], f32)
            nc.gpsimd.memset(dum[:], 0.0)
            nc.scalar.activation(dum[:], dum[:], mybir.ActivationFunctionType.Exp)
            lncb = nc.alloc_sbuf_tensor("lncb", [128, 1], f32)
            nc.gpsimd.memset(lncb[:], LNC)
            onesq = nc.alloc_sbuf_tensor("onesq", [128, Q], f32)
            nc.gpsimd.memset(onesq[:], 1.0)
            blockmask = nc.alloc_sbuf_tensor("blockmask", [128, Q], bf16)
            nc.gpsimd.memset(blockmask[:], 0.0)
            for q in range(Q):
                nc.gpsimd.memset(blockmask[KT * q : KT * (q + 1), q : q + 1], 1.0)
            # tap grid: value t' = k mod KT along the free axis, d = DMAX - t'
            tgrid = nc.alloc_sbuf_tensor("tgrid", [128, 128], f32)
            nc.gpsimd.iota(
                tgrid[:], pattern=[[0, Q], [1, KT]], base=0, channel_multiplier=0,
                allow_small_or_imprecise_dtypes=True,
            )
            dd = nc.alloc_sbuf_tensor("dd", [128, 128], f32)
            nc.vector.tensor_scalar(
                dd[:], tgrid[:], -1.0, float(DMAX),
                mybir.AluOpType.mult, mybir.AluOpType.add,
            )
            d2n = nc.alloc_sbuf_tensor("d2n", [128, 128], f32)
            nc.vector.scalar_tensor_tensor(
                d2n[:], dd[:], -0.5, dd[:],
                mybir.AluOpType.mult, mybir.AluOpType.mult,
            )

            # ---- sigma / w column chains (after sgw lands) ----
            rs_col = nc.alloc_sbuf_tensor("rs_col", [128, 1], f32)
            nc.vector.reciprocal(rs_col[:], sg_col)
            invs = nc.alloc_sbuf_tensor("invs", [128, 1], f32)
            nc.vector.tensor_tensor(
                invs[:], rs_col[:], rs_col[:], mybir.AluOpType.mult
            )
            ex_col = nc.alloc_sbuf_tensor("ex_col", [128, 1], f32)
            nc.scalar.activation(
                ex_col[:], w_col, mybir.ActivationFunctionType.Exp
            )
            # expt[e, k] = c * exp(-d(k)^2 / (2 sigma_e^2)), bf16
            expt = nc.alloc_sbuf_tensor("expt", [128, 128], bf16)
            nc.scalar.activation(
                expt[:], d2n[:], mybir.ActivationFunctionType.Exp,
                bias=lncb[:], scale=invs[:],
            )
            # a_e = exp(w_e) / sigma_e (unnormalized; 1/Z folded in at the end)
            a = nc.alloc_sbuf_tensor("a", [128, 1], bf16)
            nc.vector.tensor_tensor(
                a[:], ex_col[:], rs_col[:], mybir.AluOpType.mult
            )

            # ---- Z = sum_e exp(w_e), replicated onto 4 partitions ----
            z4 = nc.alloc_psum_tensor("z4", [Q, 1], f32)
            nc.tensor.matmul(z4[:], onesq[:], ex_col[:], start=True, stop=True)
            rz4 = nc.alloc_sbuf_tensor("rz4", [Q, 1], f32)
            nc.vector.reciprocal(rz4[:], z4[:])

            # ---- ghat column, replicated 4x via the tiled tap grid ----
            ghat_ps = nc.alloc_psum_tensor("ghat_ps", [128, 1], f32)
            nc.tensor.matmul(ghat_ps[:], expt[:], a[:], start=True, stop=True)
            gblk = nc.alloc_sbuf_tensor("gblk", [128, Q], bf16)
            nc.vector.tensor_scalar_mul(gblk[:], blockmask[:], ghat_ps[:, 0:1])

            # ---- binarize mask windows (bf16 0/1) ----
            mask = nc.alloc_sbuf_tensor("mask", [128, JW], bf16)
            nc.vector.tensor_scalar(
                mask[:], mw[:], 0.5, None, mybir.AluOpType.is_gt
            )

            # ---- the conv: one matmul [128,4]^T @ [128,256] ----
            out_ps = nc.alloc_psum_tensor("out_ps", [Q, JW], f32)
            nc.tensor.matmul(out_ps[:], gblk[:], mask[:], start=True, stop=True)

            # ---- normalize + add noise in the single PSUM evacuation ----
            out_sb = nc.alloc_sbuf_tensor("out_sb", [Q, JW], f32)
            nc.vector.scalar_tensor_tensor(
                out_sb[:], out_ps[:], rz4[:], wn[:],
                mybir.AluOpType.mult, mybir.AluOpType.add,
            )
            nc.sync.dma_start(out_d[:], out_sb[:])

    nc.compile()
    return nc


def kernel(X, sigma, weight, noise):
    global _compiled
    from concourse.bass_utils import run_bass_kernel_spmd

    X = np.ascontiguousarray(np.asarray(X, dtype=np.float32)).reshape(1, T)
    sigma = np.ascontiguousarray(np.asarray(sigma, dtype=np.float32)).reshape(E)
    weight = np.ascontiguousarray(np.asarray(weight, dtype=np.float32)).reshape(1, E)
    noise = np.ascontiguousarray(np.asarray(noise, dtype=np.float32)).reshape(1, T)

    if _compiled is None:
        _compiled = _build()
    nc = _compiled

    # Xpad[16 + i] = X[i]; window row k=32q+t' of core c starts at
    # Xpad[c*1024 + 256q + t'] (all-positive strides; d = 15 - t')
    Xpad = np.zeros(16 + T + DMAX, dtype=np.float32)
    Xpad[16 : 16 + T] = X[0]
    sgw = np.ascontiguousarray(
        np.stack([sigma, weight[0]], axis=1), dtype=np.float32
    )
    in_maps = []
    for c in range(N_CORES):
        mw = np.ascontiguousarray(
            np.lib.stride_tricks.as_strided(
                Xpad[c * TJ :], shape=(Q, KT, JW), strides=(4 * JW, 4, 4)
            ).reshape(128, JW)
        )
        wn = np.ascontiguousarray(
            noise[0, c * TJ : (c + 1) * TJ].reshape(Q, JW)
        )
        in_maps.append({"mw": mw, "sgw": sgw, "wn": wn})

    res = run_bass_kernel_spmd(nc, in_maps, core_ids=list(range(N_CORES)))
    out = np.empty((1, T), dtype=np.float32)
    for c in range(N_CORES):
        out[0, c * TJ : (c + 1) * TJ] = res.results[c]["out"].reshape(-1)
    return out
